# revision 17
# baseline (speedup 1.0000x reference)
"""Trainium2 Bass kernel for nn_ExpertPrefetchHead (MoE routing head).

Computes, for L=20 routing layers over B*K=8192 tokens:
    shared = gelu(x @ Wd^T) @ Wu^T                       (rank-512 shared projection)
    a_l    = gelu(x @ Ad_l^T)                            (rank-64 adapters)
    h_l    = shared + a_l @ Au_l^T
    logits_l = h_l @ G_l^T                               (E=256 experts)
    idx_l  = top8(logits_l)

Sharding: data-parallel over the 8192 tokens across 8 NeuronCores (1024
tokens/core); all weights replicated. Matmuls run in fp32r (full-rate PE,
inputs RNE-rounded to 11-bit mantissa by HW). Top-8 uses the DVE
max8/find_index8 instructions, which reproduce jax.lax.top_k ordering and
tie-breaks.
"""
import numpy as np

import concourse.bass as bass
import concourse.mybir as mybir
from concourse import bacc
from concourse.tile import TileContext
from concourse.bass_utils import run_bass_kernel_spmd

F32 = mybir.dt.float32
F32R = mybir.dt.float32r
U32 = mybir.dt.uint32
GELU = mybir.ActivationFunctionType.Gelu
COPY = mybir.ActivationFunctionType.Copy

# problem dims
L = 20
H = 2048
R = 512
A = 64
E = 256
TOPK = 8
B_DIM = 4
K_DIM = 2048
N_CORES = 8
T = (B_DIM * K_DIM) // N_CORES   # tokens per core = 1024

HC = H // 128   # 16 h-chunks
RC = R // 128   # 4 r-chunks


def build(t=T, n_layers=L):
    """Build the single-core SPMD program. t tokens, n_layers layers."""
    assert t % 256 == 0
    th_sz = t // 2           # t-half size (moving free dim of most matmuls)
    n_tt = th_sz // 128      # 128-token tiles per t-half
    n_pairs = n_layers // 2

    nc = bacc.Bacc()

    # inputs (all transposed/packed on host; fp32r so they can feed matmuls)
    xT = nc.declare_dram_parameter("xT", [H, t], F32R, isOutput=False)
    wdR = nc.declare_dram_parameter("wdR", [RC, H, 128], F32R, isOutput=False)
    wuR = nc.declare_dram_parameter("wuR", [HC, R, 128], F32R, isOutput=False)
    adP = nc.declare_dram_parameter("adP", [n_pairs, HC, 128, 128], F32R, isOutput=False)
    # block-diagonal pair packing: sub 0 occupies contraction rows 0:64,
    # sub 1 rows 64:128 (zeros elsewhere) -> K=128 matmuls instead of K=64
    auP = nc.declare_dram_parameter("auP", [n_pairs, 128, HC, 2, 128], F32R, isOutput=False)
    gTd = nc.declare_dram_parameter("gTd", [n_layers, H, E], F32R, isOutput=False)
    # outputs
    logits_out = nc.declare_dram_parameter("logits", [n_layers, t, E], F32, isOutput=True)
    idx_out = nc.declare_dram_parameter("idx", [n_layers, t, TOPK], U32, isOutput=True)

    with TileContext(nc) as tc:
        persist = tc.alloc_tile_pool(name="persist", bufs=1)
        gdpool = tc.alloc_tile_pool(name="gdpool", bufs=1)
        xtpool = tc.alloc_tile_pool(name="xtpool", bufs=1)
        # persistent tiles
        sT = persist.tile([128, HC, t], F32)            # shared^T
        ga_all = persist.tile([128, n_pairs, t], F32R)  # gelu(adapter-down), 2 layers per 128 partitions
        gd = gdpool.tile([128, RC, t], F32R)            # gelu(shared-down)^T
        xt = xtpool.tile([128, HC, t], F32R)
        nc.sync.dma_start(out=xt, in_=xT.rearrange("(c p) t -> p c t", p=128))

        # ---- phase A2: all adapter-down projections (paired layers) ----
        with tc.tile_pool(name="adpool", bufs=2) as adpool, \
             tc.tile_pool(name="ps_a", bufs=4, space="PSUM") as ps_a_pool:
            for p in range(n_pairs):
                adt = adpool.tile([128, HC, 128], F32R)
                nc.sync.dma_start(out=adt, in_=adP[p].rearrange("c p a -> p c a"))
                for th in range(2):
                    ts = slice(th * th_sz, (th + 1) * th_sz)
                    ps_a = ps_a_pool.tile([128, th_sz], F32)
                    for hc in range(HC):
                        nc.tensor.matmul(
                            ps_a, lhsT=adt[:, hc, :], rhs=xt[:, hc, ts],
                            start=(hc == 0), stop=(hc == HC - 1),
                        )
                    nc.scalar.activation(ga_all[:, p, ts], ps_a, GELU)

        # ---- phase A: shared down ----
        with tc.tile_pool(name="wdpool", bufs=2) as wdpool, \
             tc.tile_pool(name="ps_d", bufs=4, space="PSUM") as ps_d_pool:
            for rc in range(RC):
                wdt = wdpool.tile([128, HC, 128], F32R)
                nc.sync.dma_start(out=wdt, in_=wdR[rc].rearrange("(c p) r -> p c r", p=128))
                for th in range(2):
                    ts = slice(th * th_sz, (th + 1) * th_sz)
                    ps_d = ps_d_pool.tile([128, th_sz], F32)
                    for hc in range(HC):
                        nc.tensor.matmul(
                            ps_d, lhsT=wdt[:, hc, :], rhs=xt[:, hc, ts],
                            start=(hc == 0), stop=(hc == HC - 1),
                        )
                    nc.scalar.activation(gd[:, rc, ts], ps_d, GELU)

        # x no longer needed: free its SBUF before the per-layer loop.
        xtpool.release()
        with tc.tile_pool(name="wupool", bufs=1) as wupool, \
             tc.tile_pool(name="ps_s", bufs=4, space="PSUM") as ps_s_pool:
            # ---- phase A (cont.): shared up ----
            wut = wupool.tile([128, HC, RC, 128], F32R)
            nc.sync.dma_start(
                out=wut,
                in_=wuR.rearrange("c (rc p) h -> p c rc h", p=128))
            for hc in range(HC):
                for th in range(2):
                    ts = slice(th * th_sz, (th + 1) * th_sz)
                    ps_s = ps_s_pool.tile([128, th_sz], F32)
                    for rc in range(RC):
                        nc.tensor.matmul(
                            ps_s, lhsT=wut[:, hc, rc, :], rhs=gd[:, rc, ts],
                            start=(rc == 0), stop=(rc == RC - 1),
                        )
                    nc.scalar.activation(sT[:, hc, ts], ps_s, COPY)
        gdpool.release()

        # ---- phase B: per-layer adapter-up + gate + top-8 ----
        with tc.tile_pool(name="gtpool", bufs=3) as gtpool, \
             tc.tile_pool(name="aupool", bufs=2) as aupool, \
             tc.tile_pool(name="htpool", bufs=4) as htpool, \
             tc.tile_pool(name="lsbpool", bufs=6) as lsbpool, \
             tc.tile_pool(name="mvpool", bufs=6) as mvpool, \
             tc.tile_pool(name="ps_o", bufs=3, space="PSUM") as ps_o_pool, \
             tc.tile_pool(name="ps_l", bufs=1, space="PSUM") as ps_l_pool:
            aut = None
            for l in range(n_layers):
                gt = gtpool.tile([128, HC, E], F32R)
                nc.sync.dma_start(out=gt, in_=gTd[l].rearrange("(c p) e -> p c e", p=128))
                if l % 2 == 0:
                    aut = aupool.tile([128, HC, 2, 128], F32R, tag="aut")
                    nc.sync.dma_start(out=aut, in_=auP[l // 2])
                sub = l % 2
                pidx = l // 2
                for th in range(2):
                    ts = slice(th * th_sz, (th + 1) * th_sz)
                    # one PSUM bank per token-tile: a matmul with start=True
                    # clears its whole bank, so accumulation groups must not
                    # share banks.
                    ps_log = [ps_l_pool.tile([128, E], F32, name=f"ps_log{tt}", tag=f"pl{tt}")
                              for tt in range(n_tt)]

                    def emit_up(hc):
                        ps_o = ps_o_pool.tile([128, th_sz], F32)
                        # K=128 block-diag matmul: only rows sub*64..sub*64+63
                        # are nonzero, selecting this layer's adapter
                        nc.tensor.matmul(
                            ps_o, lhsT=aut[:, hc, sub, :],
                            rhs=ga_all[:, pidx, ts],
                            start=True, stop=True,
                        )
                        ht = htpool.tile([128, th_sz], F32R)
                        nc.vector.tensor_add(ht, ps_o, sT[:, hc, ts])
                        return ht

                    def emit_gate(hc, ht):
                        for tt in range(n_tt):
                            nc.tensor.matmul(
                                ps_log[tt],
                                lhsT=ht[:, tt * 128:(tt + 1) * 128],
                                rhs=gt[:, hc, :],
                                start=(hc == 0), stop=(hc == HC - 1),
                            )

                    # software pipeline: up-chain leads gate by 2 h-chunks
                    pending = {}
                    for hc in range(HC):
                        pending[hc] = emit_up(hc)
                        if hc >= 2:
                            emit_gate(hc - 2, pending.pop(hc - 2))
                    for hc in (HC - 2, HC - 1):
                        emit_gate(hc, pending.pop(hc))

                    for tt in range(n_tt):
                        lsb = lsbpool.tile([128, E], F32)
                        nc.scalar.activation(lsb, ps_log[tt], COPY)
                        mv = mvpool.tile([128, TOPK], F32, tag="mv")
                        mi = mvpool.tile([128, TOPK], U32, tag="mi")
                        nc.vector.max(out=mv, in_=lsb)
                        nc.vector.max_index(out=mi, in_max=mv, in_values=lsb)
                        tok0 = th * th_sz + tt * 128
                        nc.sync.dma_start(out=logits_out[l, tok0:tok0 + 128, :], in_=lsb)
                        nc.sync.dma_start(out=idx_out[l, tok0:tok0 + 128, :], in_=mi)
        persist.release()
    return nc


# ---------------------------------------------------------------------------
# host-side data prep


def _prep_core_inputs(x, shared_down_w, shared_up_w, ad_down_w, ad_up_w, gate_w,
                      t=T, n_layers=L):
    """Shared (weight) arrays + per-core x^T shards. Returns list of in_maps."""
    n_pairs = n_layers // 2
    xf = np.ascontiguousarray(x, dtype=np.float32).reshape(B_DIM * K_DIM, H)

    wdT = np.ascontiguousarray(shared_down_w.T, dtype=np.float32)          # [H, R]
    wdR = np.ascontiguousarray(
        wdT.reshape(H, RC, 128).transpose(1, 0, 2))                        # [RC, H, 128]
    wuT = np.ascontiguousarray(shared_up_w.T, dtype=np.float32)            # [R, H]
    wuR = np.ascontiguousarray(
        wuT.reshape(R, HC, 128).transpose(1, 0, 2))                        # [HC, R, 128]

    adT = np.ascontiguousarray(
        ad_down_w[:n_layers].transpose(0, 2, 1), dtype=np.float32)         # [L, H, A]
    adP = np.empty((n_pairs, HC, 128, 128), dtype=np.float32)
    adTh = adT.reshape(n_layers, HC, 128, A)
    for p in range(n_pairs):
        adP[p, :, :, :A] = adTh[2 * p]
        adP[p, :, :, A:] = adTh[2 * p + 1]

    auT = np.ascontiguousarray(
        ad_up_w[:n_layers].transpose(0, 2, 1), dtype=np.float32)           # [L, A, H]
    auTh = auT.reshape(n_layers, A, HC, 128)
    auP = np.zeros((n_pairs, 128, HC, 2, 128), dtype=np.float32)
    for p in range(n_pairs):
        auP[p, :A, :, 0, :] = auTh[2 * p]
        auP[p, A:, :, 1, :] = auTh[2 * p + 1]

    gT = np.ascontiguousarray(
        gate_w[:n_layers].transpose(0, 2, 1), dtype=np.float32)            # [L, H, E]

    in_maps = []
    for c in range(N_CORES):
        xs = np.ascontiguousarray(xf[c * t:(c + 1) * t].T)                 # [H, t]
        in_maps.append({
            "xT": xs, "wdR": wdR, "wuR": wuR, "adP": adP, "auP": auP, "gTd": gT,
        })
    return in_maps


_NC_CACHE = {}


def _get_nc(t=T, n_layers=L):
    key = (t, n_layers)
    if key not in _NC_CACHE:
        nc = build(t, n_layers)
        nc.finalize()
        _NC_CACHE[key] = nc
    return _NC_CACHE[key]


def run_cores(inputs, trace=False, t=T, n_layers=L):
    """Run on all 8 cores; returns (indices, logits, BassKernelResults)."""
    in_maps = _prep_core_inputs(
        inputs["x"], inputs["shared_down_w"], inputs["shared_up_w"],
        inputs["ad_down_w"], inputs["ad_up_w"], inputs["gate_w"],
        t=t, n_layers=n_layers)
    nc = _get_nc(t, n_layers)
    res = run_bass_kernel_spmd(nc, in_maps, core_ids=list(range(N_CORES)), trace=trace)
    logits = np.concatenate([res.results[c]["logits"] for c in range(N_CORES)], axis=1)
    idx = np.concatenate([res.results[c]["idx"] for c in range(N_CORES)], axis=1)
    logits = logits.reshape(n_layers, B_DIM, K_DIM, E)
    idx = idx.astype(np.int32).reshape(n_layers, B_DIM, K_DIM, TOPK)
    return idx, logits, res


def kernel(**inputs):
    idx, logits, _ = run_cores(inputs, trace=False)
    return idx, logits


# revision 19
# speedup vs baseline: 1.0976x; 1.0976x over previous
"""Trainium2 Bass kernel for nn_ExpertPrefetchHead (MoE routing head).

Computes, for L=20 routing layers over B*K=8192 tokens:
    shared = gelu(x @ Wd^T) @ Wu^T                       (rank-512 shared projection)
    a_l    = gelu(x @ Ad_l^T)                            (rank-64 adapters)
    h_l    = shared + a_l @ Au_l^T
    logits_l = h_l @ G_l^T                               (E=256 experts)
    idx_l  = top8(logits_l)

Sharding: data-parallel over the 8192 tokens across 8 NeuronCores (1024
tokens/core); all weights replicated. Matmuls run in fp32r (full-rate PE,
inputs RNE-rounded to 11-bit mantissa by HW). Top-8 uses the DVE
max8/find_index8 instructions, which reproduce jax.lax.top_k ordering and
tie-breaks.
"""
import numpy as np

import concourse.bass as bass
import concourse.mybir as mybir
from concourse import bacc
from concourse.tile import TileContext
from concourse.bass_utils import run_bass_kernel_spmd

F32 = mybir.dt.float32
F32R = mybir.dt.float32r
U32 = mybir.dt.uint32
GELU = mybir.ActivationFunctionType.Gelu
COPY = mybir.ActivationFunctionType.Copy

# problem dims
L = 20
H = 2048
R = 512
A = 64
E = 256
TOPK = 8
B_DIM = 4
K_DIM = 2048
N_CORES = 8
T = (B_DIM * K_DIM) // N_CORES   # tokens per core = 1024

HC = H // 128   # 16 h-chunks
RC = R // 128   # 4 r-chunks


def build(t=T, n_layers=L):
    """Build the single-core SPMD program. t tokens, n_layers layers."""
    assert t % 256 == 0
    th_sz = t // 2           # t-half size (moving free dim of most matmuls)
    n_tt = th_sz // 128      # 128-token tiles per t-half
    n_pairs = n_layers // 2

    nc = bacc.Bacc()

    # inputs (all transposed/packed on host; fp32r so they can feed matmuls)
    xT = nc.declare_dram_parameter("xT", [H, t], F32R, isOutput=False)
    wdR = nc.declare_dram_parameter("wdR", [RC, H, 128], F32R, isOutput=False)
    wuR = nc.declare_dram_parameter("wuR", [HC, R, 128], F32R, isOutput=False)
    adP = nc.declare_dram_parameter("adP", [n_pairs, HC, 128, 128], F32R, isOutput=False)
    # block-diagonal pair packing: sub 0 occupies contraction rows 0:64,
    # sub 1 rows 64:128 (zeros elsewhere) -> K=128 matmuls instead of K=64
    auP = nc.declare_dram_parameter("auP", [n_pairs, 128, HC, 2, 128], F32R, isOutput=False)
    gTd = nc.declare_dram_parameter("gTd", [n_layers, H, E], F32R, isOutput=False)
    # outputs
    logits_out = nc.declare_dram_parameter("logits", [n_layers, t, E], F32, isOutput=True)
    idx_out = nc.declare_dram_parameter("idx", [n_layers, t, TOPK], U32, isOutput=True)

    with TileContext(nc) as tc:
        persist = tc.alloc_tile_pool(name="persist", bufs=1)
        gdpool = tc.alloc_tile_pool(name="gdpool", bufs=1)
        xtpool = tc.alloc_tile_pool(name="xtpool", bufs=1)
        # persistent tiles
        sT = persist.tile([128, HC, t], F32)            # shared^T
        ga_all = persist.tile([128, n_pairs, t], F32R)  # gelu(adapter-down), 2 layers per 128 partitions
        gd = gdpool.tile([128, RC, t], F32R)            # gelu(shared-down)^T
        xt = xtpool.tile([128, HC, t], F32R)
        nc.sync.dma_start(out=xt, in_=xT.rearrange("(c p) t -> p c t", p=128))

        # ---- phase A2: all adapter-down projections (paired layers) ----
        with tc.tile_pool(name="adpool", bufs=2) as adpool, \
             tc.tile_pool(name="ps_a", bufs=4, space="PSUM") as ps_a_pool:
            for p in range(n_pairs):
                adt = adpool.tile([128, HC, 128], F32R)
                nc.sync.dma_start(out=adt, in_=adP[p].rearrange("c p a -> p c a"))
                for th in range(2):
                    ts = slice(th * th_sz, (th + 1) * th_sz)
                    ps_a = ps_a_pool.tile([128, th_sz], F32)
                    for hc in range(HC):
                        nc.tensor.matmul(
                            ps_a, lhsT=adt[:, hc, :], rhs=xt[:, hc, ts],
                            start=(hc == 0), stop=(hc == HC - 1),
                        )
                    nc.scalar.activation(ga_all[:, p, ts], ps_a, GELU)

        # ---- phase A: shared down ----
        with tc.tile_pool(name="wdpool", bufs=2) as wdpool, \
             tc.tile_pool(name="ps_d", bufs=4, space="PSUM") as ps_d_pool:
            for rc in range(RC):
                wdt = wdpool.tile([128, HC, 128], F32R)
                nc.sync.dma_start(out=wdt, in_=wdR[rc].rearrange("(c p) r -> p c r", p=128))
                for th in range(2):
                    ts = slice(th * th_sz, (th + 1) * th_sz)
                    ps_d = ps_d_pool.tile([128, th_sz], F32)
                    for hc in range(HC):
                        nc.tensor.matmul(
                            ps_d, lhsT=wdt[:, hc, :], rhs=xt[:, hc, ts],
                            start=(hc == 0), stop=(hc == HC - 1),
                        )
                    nc.scalar.activation(gd[:, rc, ts], ps_d, GELU)

        # x no longer needed: free its SBUF before the per-layer loop.
        xtpool.release()
        with tc.tile_pool(name="wupool", bufs=1) as wupool, \
             tc.tile_pool(name="ps_s", bufs=4, space="PSUM") as ps_s_pool:
            # ---- phase A (cont.): shared up ----
            wut = wupool.tile([128, HC, RC, 128], F32R)
            nc.sync.dma_start(
                out=wut,
                in_=wuR.rearrange("c (rc p) h -> p c rc h", p=128))
            for hc in range(HC):
                for th in range(2):
                    ts = slice(th * th_sz, (th + 1) * th_sz)
                    ps_s = ps_s_pool.tile([128, th_sz], F32)
                    for rc in range(RC):
                        nc.tensor.matmul(
                            ps_s, lhsT=wut[:, hc, rc, :], rhs=gd[:, rc, ts],
                            start=(rc == 0), stop=(rc == RC - 1),
                        )
                    nc.scalar.activation(sT[:, hc, ts], ps_s, COPY)
        gdpool.release()

        # ---- phase B: per-layer adapter-up + gate + top-8 ----
        with tc.tile_pool(name="gtpool", bufs=3) as gtpool, \
             tc.tile_pool(name="aupool", bufs=2) as aupool, \
             tc.tile_pool(name="htpool", bufs=4) as htpool, \
             tc.tile_pool(name="osbpool", bufs=4) as osbpool, \
             tc.tile_pool(name="lsbpool", bufs=6) as lsbpool, \
             tc.tile_pool(name="mvpool", bufs=6) as mvpool, \
             tc.tile_pool(name="ps_o", bufs=3, space="PSUM") as ps_o_pool, \
             tc.tile_pool(name="ps_l", bufs=1, space="PSUM") as ps_l_pool:
            aut = None
            for l in range(n_layers):
                gt = gtpool.tile([128, HC, E], F32R)
                nc.sync.dma_start(out=gt, in_=gTd[l].rearrange("(c p) e -> p c e", p=128))
                if l % 2 == 0:
                    aut = aupool.tile([128, HC, 2, 128], F32R, tag="aut")
                    nc.sync.dma_start(out=aut, in_=auP[l // 2])
                sub = l % 2
                pidx = l // 2
                for th in range(2):
                    ts = slice(th * th_sz, (th + 1) * th_sz)
                    # one PSUM bank per token-tile: a matmul with start=True
                    # clears its whole bank, so accumulation groups must not
                    # share banks.
                    ps_log = [ps_l_pool.tile([128, E], F32, name=f"ps_log{tt}", tag=f"pl{tt}")
                              for tt in range(n_tt)]

                    def emit_up(hc):
                        ps_o = ps_o_pool.tile([128, th_sz], F32)
                        # K=128 block-diag matmul: only rows sub*64..sub*64+63
                        # are nonzero, selecting this layer's adapter
                        nc.tensor.matmul(
                            ps_o, lhsT=aut[:, hc, sub, :],
                            rhs=ga_all[:, pidx, ts],
                            start=True, stop=True,
                        )
                        osb = osbpool.tile([128, th_sz], F32)
                        nc.scalar.activation(osb, ps_o, COPY)
                        ht = htpool.tile([128, th_sz], F32R)
                        nc.vector.tensor_add(ht, osb, sT[:, hc, ts])
                        return ht

                    def emit_gate(hc, ht):
                        for tt in range(n_tt):
                            nc.tensor.matmul(
                                ps_log[tt],
                                lhsT=ht[:, tt * 128:(tt + 1) * 128],
                                rhs=gt[:, hc, :],
                                start=(hc == 0), stop=(hc == HC - 1),
                            )

                    # software pipeline: up-chain leads gate by 2 h-chunks
                    pending = {}
                    for hc in range(HC):
                        pending[hc] = emit_up(hc)
                        if hc >= 2:
                            emit_gate(hc - 2, pending.pop(hc - 2))
                    for hc in (HC - 2, HC - 1):
                        emit_gate(hc, pending.pop(hc))

                    for tt in range(n_tt):
                        lsb = lsbpool.tile([128, E], F32)
                        nc.scalar.activation(lsb, ps_log[tt], COPY)
                        mv = mvpool.tile([128, TOPK], F32, tag="mv")
                        mi = mvpool.tile([128, TOPK], U32, tag="mi")
                        nc.vector.max(out=mv, in_=lsb)
                        nc.vector.max_index(out=mi, in_max=mv, in_values=lsb)
                        tok0 = th * th_sz + tt * 128
                        nc.sync.dma_start(out=logits_out[l, tok0:tok0 + 128, :], in_=lsb)
                        nc.sync.dma_start(out=idx_out[l, tok0:tok0 + 128, :], in_=mi)
        persist.release()
    return nc


# ---------------------------------------------------------------------------
# host-side data prep


def _prep_core_inputs(x, shared_down_w, shared_up_w, ad_down_w, ad_up_w, gate_w,
                      t=T, n_layers=L):
    """Shared (weight) arrays + per-core x^T shards. Returns list of in_maps."""
    n_pairs = n_layers // 2
    xf = np.ascontiguousarray(x, dtype=np.float32).reshape(B_DIM * K_DIM, H)

    wdT = np.ascontiguousarray(shared_down_w.T, dtype=np.float32)          # [H, R]
    wdR = np.ascontiguousarray(
        wdT.reshape(H, RC, 128).transpose(1, 0, 2))                        # [RC, H, 128]
    wuT = np.ascontiguousarray(shared_up_w.T, dtype=np.float32)            # [R, H]
    wuR = np.ascontiguousarray(
        wuT.reshape(R, HC, 128).transpose(1, 0, 2))                        # [HC, R, 128]

    adT = np.ascontiguousarray(
        ad_down_w[:n_layers].transpose(0, 2, 1), dtype=np.float32)         # [L, H, A]
    adP = np.empty((n_pairs, HC, 128, 128), dtype=np.float32)
    adTh = adT.reshape(n_layers, HC, 128, A)
    for p in range(n_pairs):
        adP[p, :, :, :A] = adTh[2 * p]
        adP[p, :, :, A:] = adTh[2 * p + 1]

    auT = np.ascontiguousarray(
        ad_up_w[:n_layers].transpose(0, 2, 1), dtype=np.float32)           # [L, A, H]
    auTh = auT.reshape(n_layers, A, HC, 128)
    auP = np.zeros((n_pairs, 128, HC, 2, 128), dtype=np.float32)
    for p in range(n_pairs):
        auP[p, :A, :, 0, :] = auTh[2 * p]
        auP[p, A:, :, 1, :] = auTh[2 * p + 1]

    gT = np.ascontiguousarray(
        gate_w[:n_layers].transpose(0, 2, 1), dtype=np.float32)            # [L, H, E]

    in_maps = []
    for c in range(N_CORES):
        xs = np.ascontiguousarray(xf[c * t:(c + 1) * t].T)                 # [H, t]
        in_maps.append({
            "xT": xs, "wdR": wdR, "wuR": wuR, "adP": adP, "auP": auP, "gTd": gT,
        })
    return in_maps


_NC_CACHE = {}


def _get_nc(t=T, n_layers=L):
    key = (t, n_layers)
    if key not in _NC_CACHE:
        nc = build(t, n_layers)
        nc.finalize()
        _NC_CACHE[key] = nc
    return _NC_CACHE[key]


def run_cores(inputs, trace=False, t=T, n_layers=L):
    """Run on all 8 cores; returns (indices, logits, BassKernelResults)."""
    in_maps = _prep_core_inputs(
        inputs["x"], inputs["shared_down_w"], inputs["shared_up_w"],
        inputs["ad_down_w"], inputs["ad_up_w"], inputs["gate_w"],
        t=t, n_layers=n_layers)
    nc = _get_nc(t, n_layers)
    res = run_bass_kernel_spmd(nc, in_maps, core_ids=list(range(N_CORES)), trace=trace)
    logits = np.concatenate([res.results[c]["logits"] for c in range(N_CORES)], axis=1)
    idx = np.concatenate([res.results[c]["idx"] for c in range(N_CORES)], axis=1)
    logits = logits.reshape(n_layers, B_DIM, K_DIM, E)
    idx = idx.astype(np.int32).reshape(n_layers, B_DIM, K_DIM, TOPK)
    return idx, logits, res


def kernel(**inputs):
    idx, logits, _ = run_cores(inputs, trace=False)
    return idx, logits


# revision 27
# speedup vs baseline: 1.2933x; 1.1783x over previous
"""Trainium2 Bass kernel for nn_ExpertPrefetchHead (MoE routing head).

Computes, for L=20 routing layers over B*K=8192 tokens:
    shared = gelu(x @ Wd^T) @ Wu^T                       (rank-512 shared projection)
    a_l    = gelu(x @ Ad_l^T)                            (rank-64 adapters)
    h_l    = shared + a_l @ Au_l^T
    logits_l = h_l @ G_l^T                               (E=256 experts)
    idx_l  = top8(logits_l)

Sharding: data-parallel over the 8192 tokens across 8 NeuronCores (1024
tokens/core); all weights replicated. Matmuls run in fp32r (full-rate PE,
inputs RNE-rounded to 11-bit mantissa by HW). Top-8 uses the DVE
max8/find_index8 instructions, which reproduce jax.lax.top_k ordering and
tie-breaks.
"""
import numpy as np

import concourse.bass as bass
import concourse.mybir as mybir
from concourse import bacc
from concourse.tile import TileContext
from concourse.bass_utils import run_bass_kernel_spmd

F32 = mybir.dt.float32
F32R = mybir.dt.float32r
U32 = mybir.dt.uint32
GELU = mybir.ActivationFunctionType.Gelu
COPY = mybir.ActivationFunctionType.Copy

# problem dims
L = 20
H = 2048
R = 512
A = 64
E = 256
TOPK = 8
B_DIM = 4
K_DIM = 2048
N_CORES = 8
T = (B_DIM * K_DIM) // N_CORES   # tokens per core = 1024

HC = H // 128   # 16 h-chunks
RC = R // 128   # 4 r-chunks


def build(t=T, n_layers=L):
    """Build the single-core SPMD program. t tokens, n_layers layers."""
    assert t % 256 == 0
    th_sz = t // 2           # t-half size (moving free dim of most matmuls)
    n_tt = th_sz // 128      # 128-token tiles per t-half
    n_pairs = n_layers // 2

    nc = bacc.Bacc()

    # inputs (all transposed/packed on host; fp32r so they can feed matmuls)
    xT = nc.declare_dram_parameter("xT", [H, t], F32R, isOutput=False)
    wdR = nc.declare_dram_parameter("wdR", [RC, H, 128], F32R, isOutput=False)
    wuR = nc.declare_dram_parameter("wuR", [HC, R, 128], F32R, isOutput=False)
    adP = nc.declare_dram_parameter("adP", [n_pairs, HC, 128, 128], F32R, isOutput=False)
    # ad_up in [h, a] chunk layout (for G~ = Au^T @ G^T), packed per pair
    auH = nc.declare_dram_parameter("auH", [n_pairs, HC, 128, 2, A], F32R, isOutput=False)
    gTd = nc.declare_dram_parameter("gTd", [n_layers, H, E], F32R, isOutput=False)
    # outputs
    logits_out = nc.declare_dram_parameter("logits", [n_layers, t, E], F32, isOutput=True)
    idx_out = nc.declare_dram_parameter("idx", [n_layers, t, TOPK], U32, isOutput=True)

    with TileContext(nc) as tc:
        persist = tc.alloc_tile_pool(name="persist", bufs=1)
        gdpool = tc.alloc_tile_pool(name="gdpool", bufs=1)
        xtpool = tc.alloc_tile_pool(name="xtpool", bufs=1)
        # persistent tiles
        sT = persist.tile([128, HC, t], F32R)           # shared^T
        ga_all = persist.tile([128, n_pairs, t], F32R)  # gelu(adapter-down), 2 layers per 128 partitions
        gd = gdpool.tile([128, RC, t], F32R)            # gelu(shared-down)^T
        xt = xtpool.tile([128, HC, t], F32R)
        nc.sync.dma_start(out=xt, in_=xT.rearrange("(c p) t -> p c t", p=128))

        # ---- phase A2: all adapter-down projections (paired layers) ----
        with tc.tile_pool(name="adpool", bufs=2) as adpool, \
             tc.tile_pool(name="ps_a", bufs=4, space="PSUM") as ps_a_pool:
            for p in range(n_pairs):
                adt = adpool.tile([128, HC, 128], F32R)
                nc.sync.dma_start(out=adt, in_=adP[p].rearrange("c p a -> p c a"))
                for th in range(2):
                    ts = slice(th * th_sz, (th + 1) * th_sz)
                    ps_a = ps_a_pool.tile([128, th_sz], F32)
                    for hc in range(HC):
                        nc.tensor.matmul(
                            ps_a, lhsT=adt[:, hc, :], rhs=xt[:, hc, ts],
                            start=(hc == 0), stop=(hc == HC - 1),
                        )
                    nc.scalar.activation(ga_all[:, p, ts], ps_a, GELU)

        # ---- phase A: shared down ----
        with tc.tile_pool(name="wdpool", bufs=2) as wdpool, \
             tc.tile_pool(name="ps_d", bufs=4, space="PSUM") as ps_d_pool:
            for rc in range(RC):
                wdt = wdpool.tile([128, HC, 128], F32R)
                nc.sync.dma_start(out=wdt, in_=wdR[rc].rearrange("(c p) r -> p c r", p=128))
                for th in range(2):
                    ts = slice(th * th_sz, (th + 1) * th_sz)
                    ps_d = ps_d_pool.tile([128, th_sz], F32)
                    for hc in range(HC):
                        nc.tensor.matmul(
                            ps_d, lhsT=wdt[:, hc, :], rhs=xt[:, hc, ts],
                            start=(hc == 0), stop=(hc == HC - 1),
                        )
                    nc.scalar.activation(gd[:, rc, ts], ps_d, GELU)

        # x no longer needed: free its SBUF before the per-layer loop.
        xtpool.release()
        with tc.tile_pool(name="wupool", bufs=1) as wupool, \
             tc.tile_pool(name="ps_s", bufs=4, space="PSUM") as ps_s_pool:
            # ---- phase A (cont.): shared up ----
            wut = wupool.tile([128, HC, RC, 128], F32R)
            nc.sync.dma_start(
                out=wut,
                in_=wuR.rearrange("c (rc p) h -> p c rc h", p=128))
            for hc in range(HC):
                for th in range(2):
                    ts = slice(th * th_sz, (th + 1) * th_sz)
                    ps_s = ps_s_pool.tile([128, th_sz], F32)
                    for rc in range(RC):
                        nc.tensor.matmul(
                            ps_s, lhsT=wut[:, hc, rc, :], rhs=gd[:, rc, ts],
                            start=(rc == 0), stop=(rc == RC - 1),
                        )
                    nc.scalar.activation(sT[:, hc, ts], ps_s, COPY)
        gdpool.release()

        # ---- phase B: per-layer gate with reassociated adapter term ----
        # logits_l = s @ G_l^T + ga_l @ (Au_l^T @ G_l^T)
        # The [A, E] product G~ is computed once per layer on the PE (1.8us),
        # then the adapter contribution folds into each token-tile's gate
        # accumulation as one extra K=128 matmul (block layout: even layer
        # in contraction rows 0:64 of g~sb, odd in 64:128, zeros elsewhere).
        n_tok_tiles = t // 128
        with tc.tile_pool(name="gtpool", bufs=3) as gtpool, \
             tc.tile_pool(name="aupool", bufs=2) as aupool, \
             tc.tile_pool(name="gtilpool", bufs=3) as gtilpool, \
             tc.tile_pool(name="tmppool", bufs=2) as tmppool, \
             tc.tile_pool(name="lsbpool", bufs=8) as lsbpool, \
             tc.tile_pool(name="mvpool", bufs=8) as mvpool, \
             tc.tile_pool(name="ps_g", bufs=2, space="PSUM") as ps_g_pool, \
             tc.tile_pool(name="ps_l", bufs=4, space="PSUM") as ps_l_pool:
            auh = None
            for l in range(n_layers):
                sub = l % 2
                pidx = l // 2
                gt = gtpool.tile([128, HC, E], F32R)
                nc.sync.dma_start(out=gt, in_=gTd[l].rearrange("(c p) e -> p c e", p=128))
                if sub == 0:
                    auh = aupool.tile([128, HC, 2, A], F32R, tag="auh")
                    nc.sync.dma_start(out=auh, in_=auH[pidx].rearrange("c p s a -> p c s a"))

                # G~_l = Au_l^T @ G_l^T  -> [A, E]
                ps_g = ps_g_pool.tile([A, E], F32)
                for hc in range(HC):
                    nc.tensor.matmul(
                        ps_g, lhsT=auh[:, hc, sub, :], rhs=gt[:, hc, :],
                        start=(hc == 0), stop=(hc == HC - 1),
                    )
                gtil = gtilpool.tile([128, E], F32R)
                if sub == 0:
                    nc.scalar.activation(gtil[0:A, :], ps_g, COPY)
                else:
                    gtmp = tmppool.tile([A, E], F32R)
                    nc.scalar.activation(gtmp, ps_g, COPY)
                    # partition shift 0:64 -> 64:128 via SBUF-to-SBUF DMA so the
                    # K=64 adapter matmul's operands share base partition
                    nc.sync.dma_start(out=gtil[A:2 * A, :], in_=gtmp)

                for tt in range(n_tok_tiles):
                    tsl = slice(tt * 128, (tt + 1) * 128)
                    ps_log = ps_l_pool.tile([128, E], F32)
                    for hc in range(HC):
                        nc.tensor.matmul(
                            ps_log, lhsT=sT[:, hc, tsl], rhs=gt[:, hc, :],
                            start=(hc == 0), stop=False,
                        )
                    pb = sub * A
                    nc.tensor.matmul(
                        ps_log, lhsT=ga_all[pb:pb + A, pidx, tsl],
                        rhs=gtil[pb:pb + A, :],
                        start=False, stop=True,
                    )
                    lsb = lsbpool.tile([128, E], F32)
                    nc.scalar.activation(lsb, ps_log, COPY)
                    mv = mvpool.tile([128, TOPK], F32, tag="mv")
                    mi = mvpool.tile([128, TOPK], U32, tag="mi")
                    nc.vector.max(out=mv, in_=lsb)
                    nc.vector.max_index(out=mi, in_max=mv, in_values=lsb)
                    nc.sync.dma_start(out=logits_out[l, tsl, :], in_=lsb)
                    nc.sync.dma_start(out=idx_out[l, tsl, :], in_=mi)
        persist.release()
    return nc


# ---------------------------------------------------------------------------
# host-side data prep


def _prep_core_inputs(x, shared_down_w, shared_up_w, ad_down_w, ad_up_w, gate_w,
                      t=T, n_layers=L):
    """Shared (weight) arrays + per-core x^T shards. Returns list of in_maps."""
    n_pairs = n_layers // 2
    xf = np.ascontiguousarray(x, dtype=np.float32).reshape(B_DIM * K_DIM, H)

    wdT = np.ascontiguousarray(shared_down_w.T, dtype=np.float32)          # [H, R]
    wdR = np.ascontiguousarray(
        wdT.reshape(H, RC, 128).transpose(1, 0, 2))                        # [RC, H, 128]
    wuT = np.ascontiguousarray(shared_up_w.T, dtype=np.float32)            # [R, H]
    wuR = np.ascontiguousarray(
        wuT.reshape(R, HC, 128).transpose(1, 0, 2))                        # [HC, R, 128]

    adT = np.ascontiguousarray(
        ad_down_w[:n_layers].transpose(0, 2, 1), dtype=np.float32)         # [L, H, A]
    adP = np.empty((n_pairs, HC, 128, 128), dtype=np.float32)
    adTh = adT.reshape(n_layers, HC, 128, A)
    for p in range(n_pairs):
        adP[p, :, :, :A] = adTh[2 * p]
        adP[p, :, :, A:] = adTh[2 * p + 1]

    auHh = np.asarray(ad_up_w[:n_layers], dtype=np.float32).reshape(
        n_layers, HC, 128, A)                                              # [L, HC, 128, A]
    auH = np.empty((n_pairs, HC, 128, 2, A), dtype=np.float32)
    for p in range(n_pairs):
        auH[p, :, :, 0, :] = auHh[2 * p]
        auH[p, :, :, 1, :] = auHh[2 * p + 1]
    auH = np.ascontiguousarray(auH)

    gT = np.ascontiguousarray(
        gate_w[:n_layers].transpose(0, 2, 1), dtype=np.float32)            # [L, H, E]

    in_maps = []
    for c in range(N_CORES):
        xs = np.ascontiguousarray(xf[c * t:(c + 1) * t].T)                 # [H, t]
        in_maps.append({
            "xT": xs, "wdR": wdR, "wuR": wuR, "adP": adP, "auH": auH, "gTd": gT,
        })
    return in_maps


_NC_CACHE = {}


def _get_nc(t=T, n_layers=L):
    key = (t, n_layers)
    if key not in _NC_CACHE:
        nc = build(t, n_layers)
        nc.finalize()
        _NC_CACHE[key] = nc
    return _NC_CACHE[key]


def run_cores(inputs, trace=False, t=T, n_layers=L):
    """Run on all 8 cores; returns (indices, logits, BassKernelResults)."""
    in_maps = _prep_core_inputs(
        inputs["x"], inputs["shared_down_w"], inputs["shared_up_w"],
        inputs["ad_down_w"], inputs["ad_up_w"], inputs["gate_w"],
        t=t, n_layers=n_layers)
    nc = _get_nc(t, n_layers)
    res = run_bass_kernel_spmd(nc, in_maps, core_ids=list(range(N_CORES)), trace=trace)
    logits = np.concatenate([res.results[c]["logits"] for c in range(N_CORES)], axis=1)
    idx = np.concatenate([res.results[c]["idx"] for c in range(N_CORES)], axis=1)
    logits = logits.reshape(n_layers, B_DIM, K_DIM, E)
    idx = idx.astype(np.int32).reshape(n_layers, B_DIM, K_DIM, TOPK)
    return idx, logits, res


def kernel(**inputs):
    idx, logits, _ = run_cores(inputs, trace=False)
    return idx, logits


# revision 28
# speedup vs baseline: 1.3912x; 1.0756x over previous
"""Trainium2 Bass kernel for nn_ExpertPrefetchHead (MoE routing head).

Computes, for L=20 routing layers over B*K=8192 tokens:
    shared = gelu(x @ Wd^T) @ Wu^T                       (rank-512 shared projection)
    a_l    = gelu(x @ Ad_l^T)                            (rank-64 adapters)
    h_l    = shared + a_l @ Au_l^T
    logits_l = h_l @ G_l^T                               (E=256 experts)
    idx_l  = top8(logits_l)

Sharding: data-parallel over the 8192 tokens across 8 NeuronCores (1024
tokens/core); all weights replicated. Matmuls run in fp32r (full-rate PE,
inputs RNE-rounded to 11-bit mantissa by HW). Top-8 uses the DVE
max8/find_index8 instructions, which reproduce jax.lax.top_k ordering and
tie-breaks.
"""
import numpy as np

import concourse.bass as bass
import concourse.mybir as mybir
from concourse import bacc
from concourse.tile import TileContext
from concourse.bass_utils import run_bass_kernel_spmd

F32 = mybir.dt.float32
F32R = mybir.dt.float32r
U32 = mybir.dt.uint32
GELU = mybir.ActivationFunctionType.Gelu
COPY = mybir.ActivationFunctionType.Copy

# problem dims
L = 20
H = 2048
R = 512
A = 64
E = 256
TOPK = 8
B_DIM = 4
K_DIM = 2048
N_CORES = 8
T = (B_DIM * K_DIM) // N_CORES   # tokens per core = 1024

HC = H // 128   # 16 h-chunks
RC = R // 128   # 4 r-chunks


def build(t=T, n_layers=L):
    """Build the single-core SPMD program. t tokens, n_layers layers."""
    assert t % 256 == 0
    th_sz = t // 2           # t-half size (moving free dim of most matmuls)
    n_tt = th_sz // 128      # 128-token tiles per t-half
    n_pairs = n_layers // 2

    nc = bacc.Bacc()

    # inputs (all transposed/packed on host; fp32r so they can feed matmuls)
    xT = nc.declare_dram_parameter("xT", [H, t], F32R, isOutput=False)
    wdR = nc.declare_dram_parameter("wdR", [RC, H, 128], F32R, isOutput=False)
    wuR = nc.declare_dram_parameter("wuR", [HC, R, 128], F32R, isOutput=False)
    adP = nc.declare_dram_parameter("adP", [n_pairs, HC, 128, 128], F32R, isOutput=False)
    # ad_up in [h, a] chunk layout (for G~ = Au^T @ G^T), packed per pair
    auH = nc.declare_dram_parameter("auH", [n_pairs, HC, 128, 2, A], F32R, isOutput=False)
    gTd = nc.declare_dram_parameter("gTd", [n_layers, H, E], F32R, isOutput=False)
    # outputs
    logits_out = nc.declare_dram_parameter("logits", [n_layers, t, E], F32, isOutput=True)
    idx_out = nc.declare_dram_parameter("idx", [n_layers, t, TOPK], U32, isOutput=True)

    with TileContext(nc) as tc:
        pool_ga = tc.alloc_tile_pool(name="pool_ga", bufs=1)
        pool_gd = tc.alloc_tile_pool(name="pool_gd", bufs=1)
        pool_xt = tc.alloc_tile_pool(name="pool_xt", bufs=1)
        ga_all = pool_ga.tile([128, n_pairs, t], F32R)  # gelu(adapter-down), 2 layers per 128 partitions
        gd = pool_gd.tile([128, RC, t], F32R)           # gelu(shared-down)^T
        xt = pool_xt.tile([128, HC, t], F32R)
        # chunked load so phase A2 can start on the first quarter
        for q in range(4):
            nc.sync.dma_start(
                out=xt[:, 4 * q:4 * (q + 1), :],
                in_=xT.rearrange("(c p) t -> p c t", p=128)[:, 4 * q:4 * (q + 1), :])

        # ---- phase A2: all adapter-down projections (paired layers) ----
        # ---- phase A-down: shared down projection ----
        adpool = tc.alloc_tile_pool(name="adpool", bufs=3)
        wdpool = tc.alloc_tile_pool(name="wdpool", bufs=3)
        with tc.tile_pool(name="ps_a", bufs=4, space="PSUM") as ps_a_pool:
            for p in range(n_pairs):
                adt = adpool.tile([128, HC, 128], F32R)
                nc.sync.dma_start(out=adt, in_=adP[p].rearrange("c p a -> p c a"))
                for th in range(2):
                    ts = slice(th * th_sz, (th + 1) * th_sz)
                    ps_a = ps_a_pool.tile([128, th_sz], F32)
                    for hc in range(HC):
                        nc.tensor.matmul(
                            ps_a, lhsT=adt[:, hc, :], rhs=xt[:, hc, ts],
                            start=(hc == 0), stop=(hc == HC - 1),
                        )
                    nc.scalar.activation(ga_all[:, p, ts], ps_a, GELU)
            for rc in range(RC):
                wdt = wdpool.tile([128, HC, 128], F32R)
                nc.sync.dma_start(out=wdt, in_=wdR[rc].rearrange("(c p) r -> p c r", p=128))
                for th in range(2):
                    ts = slice(th * th_sz, (th + 1) * th_sz)
                    ps_d = ps_a_pool.tile([128, th_sz], F32, tag="ps_d")
                    for hc in range(HC):
                        nc.tensor.matmul(
                            ps_d, lhsT=wdt[:, hc, :], rhs=xt[:, hc, ts],
                            start=(hc == 0), stop=(hc == HC - 1),
                        )
                    nc.scalar.activation(gd[:, rc, ts], ps_d, GELU)
        wdpool.release()
        adpool.release()
        # x no longer needed: free its SBUF; sT + phase-B pools take its place
        xtpool_released = True
        pool_xt.release()

        pool_sT = tc.alloc_tile_pool(name="pool_sT", bufs=1)
        sT = pool_sT.tile([128, HC, t], F32R)           # shared^T

        # phase-B SBUF pools allocated *before* the shared-up scratch so their
        # DMAs (first gate weights) can prefetch during phase A-up
        gtpool = tc.alloc_tile_pool(name="gtpool", bufs=3)
        aupool = tc.alloc_tile_pool(name="aupool", bufs=2)
        gtilpool = tc.alloc_tile_pool(name="gtilpool", bufs=3)
        tmppool = tc.alloc_tile_pool(name="tmppool", bufs=2)
        lsbpool = tc.alloc_tile_pool(name="lsbpool", bufs=8)
        mvpool = tc.alloc_tile_pool(name="mvpool", bufs=8)

        with tc.tile_pool(name="wupool", bufs=3) as wupool, \
             tc.tile_pool(name="ps_s", bufs=4, space="PSUM") as ps_s_pool:
            # ---- phase A-up: shared up projection ----
            for hc in range(HC):
                wut = wupool.tile([128, RC, 128], F32R)
                nc.sync.dma_start(out=wut, in_=wuR[hc].rearrange("(c p) h -> p c h", p=128))
                for th in range(2):
                    ts = slice(th * th_sz, (th + 1) * th_sz)
                    ps_s = ps_s_pool.tile([128, th_sz], F32)
                    for rc in range(RC):
                        nc.tensor.matmul(
                            ps_s, lhsT=wut[:, rc, :], rhs=gd[:, rc, ts],
                            start=(rc == 0), stop=(rc == RC - 1),
                        )
                    nc.scalar.activation(sT[:, hc, ts], ps_s, COPY)

        # ---- phase B: per-layer gate with reassociated adapter term ----
        # logits_l = s @ G_l^T + ga_l @ (Au_l^T @ G_l^T)
        # The [A, E] product G~ is computed once per layer on the PE (~2us),
        # then the adapter contribution folds into each token-tile's gate
        # accumulation as one extra K=128 matmul (G~ block-placed: even layer
        # in contraction rows 0:64, odd in 64:128, zeros in the other half).
        n_tok_tiles = t // 128
        with tc.tile_pool(name="ps_g", bufs=2, space="PSUM") as ps_g_pool, \
             tc.tile_pool(name="ps_l", bufs=4, space="PSUM") as ps_l_pool:
            auh = None
            for l in range(n_layers):
                sub = l % 2
                pidx = l // 2
                gt = gtpool.tile([128, HC, E], F32R)
                nc.sync.dma_start(out=gt, in_=gTd[l].rearrange("(c p) e -> p c e", p=128))
                if sub == 0:
                    auh = aupool.tile([128, HC, 2, A], F32R, tag="auh")
                    nc.sync.dma_start(out=auh, in_=auH[pidx].rearrange("c p s a -> p c s a"))

                # G~_l = Au_l^T @ G_l^T  -> [A, E]
                ps_g = ps_g_pool.tile([A, E], F32)
                for hc in range(HC):
                    nc.tensor.matmul(
                        ps_g, lhsT=auh[:, hc, sub, :], rhs=gt[:, hc, :],
                        start=(hc == 0), stop=(hc == HC - 1),
                    )
                gtil = gtilpool.tile([128, E], F32R)
                if sub == 0:
                    nc.scalar.activation(gtil[0:A, :], ps_g, COPY)
                    # zero the other contraction half (scale=0 copy; input is
                    # just any tile covering partitions 64:128)
                    nc.scalar.activation(gtil[A:2 * A, :], gt[A:2 * A, 0, :], COPY, scale=0.0)
                else:
                    gtmp = tmppool.tile([A, E], F32R)
                    nc.scalar.activation(gtmp, ps_g, COPY)
                    # partition shift 0:64 -> 64:128 via SBUF-to-SBUF DMA
                    nc.sync.dma_start(out=gtil[A:2 * A, :], in_=gtmp)
                    nc.scalar.activation(gtil[0:A, :], gt[0:A, 0, :], COPY, scale=0.0)

                for tt in range(n_tok_tiles):
                    tsl = slice(tt * 128, (tt + 1) * 128)
                    ps_log = ps_l_pool.tile([128, E], F32)
                    for hc in range(HC):
                        nc.tensor.matmul(
                            ps_log, lhsT=sT[:, hc, tsl], rhs=gt[:, hc, :],
                            start=(hc == 0), stop=False,
                        )
                    nc.tensor.matmul(
                        ps_log, lhsT=ga_all[:, pidx, tsl], rhs=gtil,
                        start=False, stop=True,
                    )
                    lsb = lsbpool.tile([128, E], F32)
                    nc.scalar.activation(lsb, ps_log, COPY)
                    mv = mvpool.tile([128, TOPK], F32, tag="mv")
                    mi = mvpool.tile([128, TOPK], U32, tag="mi")
                    nc.vector.max(out=mv, in_=lsb)
                    nc.vector.max_index(out=mi, in_max=mv, in_values=lsb)
                    nc.sync.dma_start(out=logits_out[l, tsl, :], in_=lsb)
                    nc.sync.dma_start(out=idx_out[l, tsl, :], in_=mi)
        for pool in (mvpool, lsbpool, tmppool, gtilpool, aupool, gtpool, pool_sT,
                     pool_gd, pool_ga):
            pool.release()
    return nc


# ---------------------------------------------------------------------------
# host-side data prep


def _prep_core_inputs(x, shared_down_w, shared_up_w, ad_down_w, ad_up_w, gate_w,
                      t=T, n_layers=L):
    """Shared (weight) arrays + per-core x^T shards. Returns list of in_maps."""
    n_pairs = n_layers // 2
    xf = np.ascontiguousarray(x, dtype=np.float32).reshape(B_DIM * K_DIM, H)

    wdT = np.ascontiguousarray(shared_down_w.T, dtype=np.float32)          # [H, R]
    wdR = np.ascontiguousarray(
        wdT.reshape(H, RC, 128).transpose(1, 0, 2))                        # [RC, H, 128]
    wuT = np.ascontiguousarray(shared_up_w.T, dtype=np.float32)            # [R, H]
    wuR = np.ascontiguousarray(
        wuT.reshape(R, HC, 128).transpose(1, 0, 2))                        # [HC, R, 128]

    adT = np.ascontiguousarray(
        ad_down_w[:n_layers].transpose(0, 2, 1), dtype=np.float32)         # [L, H, A]
    adP = np.empty((n_pairs, HC, 128, 128), dtype=np.float32)
    adTh = adT.reshape(n_layers, HC, 128, A)
    for p in range(n_pairs):
        adP[p, :, :, :A] = adTh[2 * p]
        adP[p, :, :, A:] = adTh[2 * p + 1]

    auHh = np.asarray(ad_up_w[:n_layers], dtype=np.float32).reshape(
        n_layers, HC, 128, A)                                              # [L, HC, 128, A]
    auH = np.empty((n_pairs, HC, 128, 2, A), dtype=np.float32)
    for p in range(n_pairs):
        auH[p, :, :, 0, :] = auHh[2 * p]
        auH[p, :, :, 1, :] = auHh[2 * p + 1]
    auH = np.ascontiguousarray(auH)

    gT = np.ascontiguousarray(
        gate_w[:n_layers].transpose(0, 2, 1), dtype=np.float32)            # [L, H, E]

    in_maps = []
    for c in range(N_CORES):
        xs = np.ascontiguousarray(xf[c * t:(c + 1) * t].T)                 # [H, t]
        in_maps.append({
            "xT": xs, "wdR": wdR, "wuR": wuR, "adP": adP, "auH": auH, "gTd": gT,
        })
    return in_maps


_NC_CACHE = {}


def _get_nc(t=T, n_layers=L):
    key = (t, n_layers)
    if key not in _NC_CACHE:
        nc = build(t, n_layers)
        nc.finalize()
        _NC_CACHE[key] = nc
    return _NC_CACHE[key]


def run_cores(inputs, trace=False, t=T, n_layers=L):
    """Run on all 8 cores; returns (indices, logits, BassKernelResults)."""
    in_maps = _prep_core_inputs(
        inputs["x"], inputs["shared_down_w"], inputs["shared_up_w"],
        inputs["ad_down_w"], inputs["ad_up_w"], inputs["gate_w"],
        t=t, n_layers=n_layers)
    nc = _get_nc(t, n_layers)
    res = run_bass_kernel_spmd(nc, in_maps, core_ids=list(range(N_CORES)), trace=trace)
    logits = np.concatenate([res.results[c]["logits"] for c in range(N_CORES)], axis=1)
    idx = np.concatenate([res.results[c]["idx"] for c in range(N_CORES)], axis=1)
    logits = logits.reshape(n_layers, B_DIM, K_DIM, E)
    idx = idx.astype(np.int32).reshape(n_layers, B_DIM, K_DIM, TOPK)
    return idx, logits, res


def kernel(**inputs):
    idx, logits, _ = run_cores(inputs, trace=False)
    return idx, logits
